# revision 1
# baseline (speedup 1.0000x reference)
"""Trainium2 Bass kernel for nn_Block (ragged transformer block).

B=2, T=2048, D=768, H=12, DH=64, FF=3072.

Sharding: 8 cores = 2 batches x 4 query-blocks of 512 tokens.
Each core computes K^T/V over the full sequence of its batch (replicated
within the 4-core batch group), and Q/attention/proj/LN/FFN for its own
512 tokens. No collectives; the host gathers the 8 disjoint output slabs.

On-chip layout is feature-major ("transposed", [feature, token]) end to
end, so every matmul chains with no transposes:
  xT -> Q^T/K^T (feat-major) and V (token-major) -> S^T = K^T.T @ Q^T
  -> exp (key mask folded into the ACT bias) -> U^T = V'.T @ expS^T
  (softmax sums via ones-bias columns baked into V') -> a^T -> proj
  -> LN1 (stats via ones-matmul, broadcast for free) -> FFN (gelu bias
  on ACT) -> LN2 -> h^T.

All matmuls run in float32r (full PE rate at N>=256, ~1.6e-4 rel err).
Padded query rows are zeroed via a host-provided row mask, matching the
reference exactly (its LN outputs are masked the same way).
"""
import sys
for _p in ("/opt/trn_rl_repo", "/root/.axon_site/_ro/trn_rl_repo"):
    if _p not in sys.path:
        sys.path.append(_p)

from contextlib import ExitStack
import numpy as np

B, T, D, H, DH, FF = 2, 2048, 768, 12, 64, 3072
M = 512            # tokens per core
DC = 6             # D / 128
FC = 24            # FF / 128
NKC = 16           # T / 128
VW = H * (DH + 1)  # 780: V' width (64 cols + 1 ones-bias col per head)
EPS = 1e-5
NEG = -1.0e9

_STATE: dict = {}


def _build_program(reps=1):
    import concourse.mybir as mybir
    import concourse.tile as tile
    from concourse import bacc

    F32 = mybir.dt.float32
    F32R = mybir.dt.float32r
    AF = mybir.ActivationFunctionType
    OP = mybir.AluOpType

    nc = bacc.Bacc("TRN2", target_bir_lowering=False, debug=False, num_devices=8)

    def din(name, shape, dt=F32R):
        return nc.dram_tensor(name, shape, dt, kind="ExternalInput").ap()

    xT = din("xT", [D, T])
    xTq = din("xTq", [D, M])
    wq = din("wq", [D, D])
    wk = din("wk", [D, D])
    bq_pc = din("bq_pc", [128, DC], F32)
    bk_pc = din("bk_pc", [128, DC], F32)
    wv = din("wv", [D, VW])
    bv = din("bv", [1, VW])
    wproj = din("wproj", [D, D])
    wfc = din("wfc", [D, FF])
    wt1 = din("wt1", [1, FF])
    wt2 = din("wt2", [1, FF])
    wout = din("wout", [FF, D])
    onesr = din("onesr", [1, M])
    ones128 = din("ones128", [128, 128])
    bprj = din("bprj", [128, DC], F32)
    bfc = din("bfc", [128, FC], F32)
    bout = din("bout", [128, DC], F32)
    l1g = din("l1g", [128, DC], F32)
    l1b = din("l1b", [128, DC], F32)
    l2g = din("l2g", [128, DC], F32)
    l2b = din("l2b", [128, DC], F32)
    vmask = din("vmask", [128, NKC], F32)
    epsc = din("epsc", [128, 1], F32)
    rowmask = din("rowmask", [128, M], F32)

    hT = nc.dram_tensor("hT", [D, M], F32, kind="ExternalOutput").ap()

    xT_r = xT.rearrange("(c p) n -> p c n", p=128)
    xTq_r = xTq.rearrange("(c p) n -> p c n", p=128)
    wq_r = wq.rearrange("(c p) n -> p c n", p=128)
    wk_r = wk.rearrange("(c p) n -> p c n", p=128)
    wv_r = wv.rearrange("(c p) n -> p c n", p=128)
    wproj_r = wproj.rearrange("(c p) n -> p c n", p=128)
    wfc_r = wfc.rearrange("(c p) n -> p c n", p=128)
    hT_r = hT.rearrange("(c p) n -> c p n", p=128)

    with tile.TileContext(nc) as tc, ExitStack() as ctx:
        const = ctx.enter_context(tc.tile_pool(name="const", bufs=1))

        def cload(name, shape, dt, src):
            t = const.tile(shape, dt, tag=name, name=name + "_t")
            nc.sync.dma_start(out=t, in_=src)
            return t

        consts = {}

        def cloads():
            consts["onesr"] = cload("onesr", [1, M], F32R, onesr)
            consts["ones128"] = cload("ones128", [128, 128], F32R, ones128)
            consts["vmask"] = cload("vmask", [128, NKC], F32, vmask)
            consts["epsc"] = cload("epsc", [128, 1], F32, epsc)
            consts["rowmask"] = cload("rowmask", [128, M], F32, rowmask)
            consts["bprj"] = cload("bprj", [128, DC], F32, bprj)
            consts["bfc"] = cload("bfc", [128, FC], F32, bfc)
            consts["bout"] = cload("bout", [128, DC], F32, bout)
            consts["l1g"] = cload("l1g", [128, DC], F32, l1g)
            consts["l1b"] = cload("l1b", [128, DC], F32, l1b)
            consts["l2g"] = cload("l2g", [128, DC], F32, l2g)
            consts["l2b"] = cload("l2b", [128, DC], F32, l2b)

        # ---------------- LN helpers (transposed layout) ----------------
        def ln_stats_bc(lnp_t, lnp_k, psum_sum, psum_ssq):
            m_bc = lnp_k.tile([128, M], F32, tag="mbc", name="mbc")
            nc.vector.tensor_scalar_mul(m_bc, psum_sum, 1.0 / D)
            mm = lnp_t.tile([128, M], F32, tag="mm", name="mm")
            nc.vector.tensor_mul(mm, m_bc, m_bc)
            var = lnp_t.tile([128, M], F32, tag="var", name="var")
            nc.vector.scalar_tensor_tensor(var, psum_ssq, 1.0 / D, mm,
                                           op0=OP.mult, op1=OP.subtract)
            sd = lnp_t.tile([128, M], F32, tag="sd", name="sd")
            nc.scalar.activation(sd, var, AF.Sqrt, bias=consts["epsc"][:, 0:1])
            rstd = lnp_k.tile([128, M], F32, tag="rstd", name="rstd")
            nc.vector.reciprocal(rstd, sd)
            return m_bc, sd, rstd

        def ln_apply_chunks(lns, y_t, m_bc, rstd, g_t, b_t, out_cb):
            for c2 in range(DC):
                t1 = lns.tile([128, M], F32, tag="t1", name="t1")
                nc.vector.tensor_sub(t1, y_t[:, c2, :].bitcast(F32), m_bc)
                t2 = lns.tile([128, M], F32, tag="t2", name="t2")
                nc.vector.tensor_mul(t2, t1, rstd)
                t3 = lns.tile([128, M], F32, tag="t3", name="t3")
                nc.vector.tensor_scalar(t3, t2, g_t[:, c2:c2 + 1],
                                        b_t[:, c2:c2 + 1],
                                        op0=OP.mult, op1=OP.add)
                out_cb(c2, t3)

        def ln_apply(y_t, psum_sum, psum_ssq, g_t, b_t, out_cb):
            with tc.tile_pool(name="lnp", bufs=1) as lnp, \
                 tc.tile_pool(name="lns", bufs=2) as lns:
                m_bc, sd, rstd = ln_stats_bc(lnp, lnp, psum_sum, psum_ssq)
                ln_apply_chunks(lns, y_t, m_bc, rstd, g_t, b_t, out_cb)

        big = ctx.enter_context(tc.tile_pool(name="big", bufs=1))
        for _rep in range(reps):
            kT_t = big.tile([128, DC, T], F32R, tag="slotL", name="kT")
            vP_t = big.tile([128, NKC, VW], F32R, tag="slotM", name="vP")
            qT_t = big.tile([128, DC, M], F32R, tag="slotS", name="qT")

            # -------- Phase 1: QKV projections --------
            with tc.tile_pool(name="p1x", bufs=2) as p1x:
                # Q^T [768, 512] (scoped pool, freed before the tb loop)
                with tc.tile_pool(name="qsc", bufs=1) as qsc, \
                     tc.tile_pool(name="qsc2", bufs=2) as qsc2, \
                     tc.tile_pool(name="psQ", bufs=2, space="PSUM") as psQ:
                    xTq_t = qsc.tile([128, DC, M], F32R, tag="xTq", name="xTq_t")
                    for dc in range(DC):
                        nc.gpsimd.dma_start(out=xTq_t[:, dc, :],
                                            in_=xTq_r[:, dc, :])
                    bq_t = cload("bq_pc", [128, DC], F32, bq_pc)
                    bk_t = cload("bk_pc", [128, DC], F32, bk_pc)
                    for qc in range(DC):
                        wq_c = qsc2.tile([128, DC, 128], F32R, tag="wqs",
                                         name="wq_c")
                        nc.sync.dma_start(out=wq_c,
                                          in_=wq_r[:, :, qc * 128:(qc + 1) * 128])
                        pq = psQ.tile([128, M], F32, tag="pq", name="pq")
                        for dc in range(DC):
                            nc.tensor.matmul(pq, wq_c[:, dc, :], xTq_t[:, dc, :],
                                             start=(dc == 0), stop=(dc == DC - 1))
                        nc.vector.tensor_scalar_add(qT_t[:, qc, :], pq,
                                                    bq_t[:, qc:qc + 1])

                cloads()
                # K^T / V' per token-block
                with tc.tile_pool(name="p1", bufs=1) as p1, \
                     tc.tile_pool(name="p1s", bufs=3) as p1s, \
                     tc.tile_pool(name="psK", bufs=4, space="PSUM") as psK, \
                     tc.tile_pool(name="psV", bufs=4, space="PSUM") as psV:
                    bv_t = wv_t = None
                    for tb in range(4):
                        xtb = p1x.tile([128, DC, M], F32R, tag="xtb",
                                       name="xtb")
                        for dc in range(DC):
                            nc.sync.dma_start(
                                out=xtb[:, dc, :],
                                in_=xT_r[:, dc, tb * M:(tb + 1) * M])
                        for kc in range(DC):
                            wk_c = p1s.tile([128, DC, 128], F32R, tag="wks",
                                            name="wk_c")
                            nc.sync.dma_start(
                                out=wk_c, in_=wk_r[:, :, kc * 128:(kc + 1) * 128])
                            pk = psK.tile([128, M], F32, tag="pk", name="pk")
                            for dc in range(DC):
                                nc.tensor.matmul(pk, wk_c[:, dc, :], xtb[:, dc, :],
                                                 start=(dc == 0),
                                                 stop=(dc == DC - 1))
                            nc.vector.tensor_scalar_add(
                                kT_t[:, kc, tb * M:(tb + 1) * M], pk,
                                bk_t[:, kc:kc + 1])
                        if tb == 0:
                            bv_t = p1.tile([1, VW], F32R, tag="bv", name="bv_t")
                            nc.sync.dma_start(out=bv_t, in_=bv)
                            wv_t = p1.tile([128, DC, VW], F32R, tag="wv",
                                           name="wv_t")
                            nc.sync.dma_start(out=wv_t, in_=wv_r)
                        for tq in range(4):
                            tci = tb * 4 + tq
                            for vb in range(2):
                                pv = psV.tile([128, VW // 2], F32, tag="pv",
                                              name="pv")
                                for dc in range(DC):
                                    nc.tensor.matmul(
                                        pv, xtb[:, dc, tq * 128:(tq + 1) * 128],
                                        wv_t[:, dc,
                                             vb * (VW // 2):(vb + 1) * (VW // 2)],
                                        start=(dc == 0), stop=False)
                                nc.tensor.matmul(
                                    pv, consts["onesr"][0:1, 0:128],
                                    bv_t[0:1, vb * (VW // 2):(vb + 1) * (VW // 2)],
                                    start=False, stop=True)
                                nc.vector.tensor_scalar_mul(
                                    vP_t[:, tci,
                                         vb * (VW // 2):(vb + 1) * (VW // 2)],
                                    pv, consts["vmask"][:, tci:tci + 1])

            # -------- Phase 2: attention (paired-chunk exp) --------
            if True:
                aT_t = big.tile([128, DC, M], F32R, tag="slotA", name="aT")
                with tc.tile_pool(name="attp", bufs=3) as attp, \
                     tc.tile_pool(name="atts", bufs=2) as atts, \
                     tc.tile_pool(name="psS", bufs=3, space="PSUM") as psS, \
                     tc.tile_pool(name="psU", bufs=2, space="PSUM") as psU:
                    for h in range(H):
                        po = (h % 2) * 64
                        chk = h // 2
                        pu = psU.tile([128, M], F32, tag="pu", name="pu")
                        for kc2 in range(NKC // 2):
                            s2 = psS.tile([128, 2, M], F32, tag="s", name="s2")
                            for j in range(2):
                                kc = kc2 * 2 + j
                                nc.tensor.matmul(
                                    s2[:, j, :],
                                    kT_t[po:po + 64, chk,
                                         kc * 128:(kc + 1) * 128],
                                    qT_t[po:po + 64, chk, :],
                                    start=True, stop=True)
                            e2 = attp.tile([128, 2, M], F32R, tag="exp",
                                           name="e2")
                            nc.scalar.activation(e2, s2, AF.Exp)
                            for j in range(2):
                                kc = kc2 * 2 + j
                                nc.tensor.matmul(
                                    pu[0:DH + 1, :],
                                    vP_t[:, kc, h * (DH + 1):(h + 1) * (DH + 1)],
                                    e2[:, j, :],
                                    start=(kc == 0), stop=(kc == NKC - 1))
                        srow = atts.tile([1, M], F32, tag="srow", name="srow")
                        nc.vector.tensor_copy(srow, pu[DH:DH + 1, :])
                        sbc = atts.tile([64, M], F32, tag="sbc", name="sbc")
                        nc.gpsimd.partition_broadcast(sbc, srow, channels=64)
                        rinv = atts.tile([64, M], F32, tag="rinv", name="rinv")
                        nc.vector.reciprocal(rinv, sbc)
                        nc.vector.tensor_mul(aT_t[po:po + 64, chk, :],
                                             pu[0:DH, :], rinv)

            # ------------ Phase 3: proj + residual + LN1 ------------
            with tc.tile_pool(name="foldp", bufs=1) as foldp:
                    nT_t = big.tile([128, DC, M], F32R, tag="slotS", name="nT")
                    with tc.tile_pool(name="p3", bufs=1) as p3, \
                         tc.tile_pool(name="p3s", bufs=2) as p3s, \
                         tc.tile_pool(name="psP", bufs=2, space="PSUM") as psP, \
                         tc.tile_pool(name="psT", bufs=1, space="PSUM") as psT:
                        wproj_t = p3.tile([128, DC, D], F32R, tag="wproj",
                                          name="wproj_t")
                        nc.sync.dma_start(out=wproj_t, in_=wproj_r)
                        y1_t = big.tile([128, DC, M], F32R, tag="slotL",
                                        name="y1")
                        psum_sum = psT.tile([128, M], F32, tag="s1",
                                            name="psum_sum")
                        psum_ssq = psT.tile([128, M], F32, tag="s2",
                                            name="psum_ssq")
                        for do in range(DC):
                            pp = psP.tile([128, M], F32, tag="pp", name="pp")
                            for di in range(DC):
                                nc.tensor.matmul(
                                    pp, wproj_t[:, di, do * 128:(do + 1) * 128],
                                    aT_t[:, di, :], start=(di == 0),
                                    stop=(di == DC - 1))
                            xr = p3s.tile([128, M], F32R, tag="xr", name="xr")
                            nc.sync.dma_start(out=xr, in_=xTq_r[:, do, :])
                            nc.vector.scalar_tensor_tensor(
                                y1_t[:, do, :], pp, consts["bprj"][:, do:do + 1],
                                xr.bitcast(F32), op0=OP.add, op1=OP.add)
                            sq = p3s.tile([128, M], F32R, tag="sq", name="sq")
                            nc.vector.tensor_mul(sq, y1_t[:, do, :].bitcast(F32),
                                                 y1_t[:, do, :].bitcast(F32))
                            nc.tensor.matmul(psum_sum, consts["ones128"], y1_t[:, do, :],
                                             start=(do == 0), stop=(do == DC - 1))
                            nc.tensor.matmul(psum_ssq, consts["ones128"], sq,
                                             start=(do == 0), stop=(do == DC - 1))

                        m_bc, sd_bc, rstd_bc = ln_stats_bc(p3, foldp,
                                                           psum_sum, psum_ssq)
                        m_row = foldp.tile([1, M], F32R, tag="mrow",
                                           name="m_row")
                        nc.vector.tensor_copy(m_row, m_bc[0:1, :])
                        isd_row = foldp.tile([1, M], F32R, tag="isdrow",
                                             name="isd_row")
                        nc.vector.tensor_copy(isd_row, sd_bc[0:1, :])
                        rm_bc = foldp.tile([128, M], F32, tag="rmbc",
                                           name="rm_bc")
                        nc.vector.tensor_mul(rm_bc, rstd_bc, consts["rowmask"])


                    # ------------ Phase 4: FFN + residual + LN2 ------------
                    wt1_t = foldp.tile([1, FF], F32R, tag="wt1", name="wt1_t")
                    nc.sync.dma_start(out=wt1_t, in_=wt1)
                    wt2_t = foldp.tile([1, FF], F32R, tag="wt2", name="wt2_t")
                    nc.sync.dma_start(out=wt2_t, in_=wt2)
                    with tc.tile_pool(name="p4a", bufs=2) as p4a, \
                         tc.tile_pool(name="p4h", bufs=2) as p4h, \
                         tc.tile_pool(name="psM", bufs=1, space="PSUM") as psM:
                        psm = [psM.tile([128, M], F32, tag=f"m{do}",
                                        name=f"psm{do}") for do in range(DC)]
                        with tc.tile_pool(name="p4w", bufs=3) as p4w, \
                             tc.tile_pool(name="p4n", bufs=2) as p4n, \
                             tc.tile_pool(name="psF", bufs=2, space="PSUM") as psF:
                            for f in range(FC):
                                wfcf = p4w.tile([128, DC, 128], F32R, tag="wfcf",
                                                name="wfcf")
                                nc.sync.dma_start(
                                    out=wfcf,
                                    in_=wfc_r[:, :, f * 128:(f + 1) * 128])
                                woutf = p4w.tile([128, D], F32R, tag="woutf",
                                                 name="woutf")
                                nc.sync.dma_start(
                                    out=woutf, in_=wout[f * 128:(f + 1) * 128, :])
                                pf = psF.tile([128, M], F32, tag="pf", name="pf")
                                for dc in range(DC):
                                    nc.tensor.matmul(pf, wfcf[:, dc, :],
                                                     y1_t[:, dc, :],
                                                     start=(dc == 0), stop=False)
                                nc.tensor.matmul(
                                    pf, wt1_t[0:1, f * 128:(f + 1) * 128],
                                    m_row, start=False, stop=False)
                                nc.tensor.matmul(
                                    pf, wt2_t[0:1, f * 128:(f + 1) * 128],
                                    isd_row, start=False, stop=True)
                                nc.vector.tensor_mul(pf, pf, rm_bc)
                                a1 = p4a.tile([128, M], F32R, tag="a1", name="a1")
                                nc.scalar.activation(a1, pf, AF.Gelu_apprx_tanh,
                                                     bias=consts["bfc"][:, f:f + 1])
                                for do in range(DC):
                                    nc.tensor.matmul(
                                        psm[do],
                                        woutf[:, do * 128:(do + 1) * 128],
                                        a1, start=(f == 0), stop=(f == FC - 1))
                                if f % 4 == 2:
                                    c2 = f // 4
                                    t1 = p4n.tile([128, M], F32, tag="t1",
                                                  name="t1")
                                    nc.vector.tensor_sub(
                                        t1, y1_t[:, c2, :].bitcast(F32), m_bc)
                                    t2 = p4n.tile([128, M], F32, tag="t2",
                                                  name="t2")
                                    nc.vector.tensor_mul(t2, t1, rstd_bc)
                                    t3 = p4n.tile([128, M], F32, tag="t3",
                                                  name="t3")
                                    nc.vector.tensor_scalar(
                                        t3, t2, consts["l1g"][:, c2:c2 + 1],
                                        consts["l1b"][:, c2:c2 + 1],
                                        op0=OP.mult, op1=OP.add)
                                    nc.vector.tensor_mul(nT_t[:, c2, :], t3,
                                                         consts["rowmask"])

                        with tc.tile_pool(name="psT2", bufs=1,
                                          space="PSUM") as psT2:
                            y2_t = big.tile([128, DC, M], F32R,
                                            tag="slotM", name="y2")
                            psum_sum2 = psT2.tile([128, M], F32, tag="s1",
                                                  name="psum_sum2")
                            psum_ssq2 = psT2.tile([128, M], F32, tag="s2",
                                                  name="psum_ssq2")
                            for do in range(DC):
                                nc.vector.scalar_tensor_tensor(
                                    y2_t[:, do, :], psm[do],
                                    consts["bout"][:, do:do + 1],
                                    nT_t[:, do, :].bitcast(F32),
                                    op0=OP.add, op1=OP.add)
                                sq = p4a.tile([128, M], F32R, tag="sq2",
                                              name="sq2")
                                nc.vector.tensor_mul(
                                    sq, y2_t[:, do, :].bitcast(F32),
                                    y2_t[:, do, :].bitcast(F32))
                                nc.tensor.matmul(psum_sum2, consts["ones128"],
                                                 y2_t[:, do, :],
                                                 start=(do == 0),
                                                 stop=(do == DC - 1))
                                nc.tensor.matmul(psum_ssq2, consts["ones128"], sq,
                                                 start=(do == 0),
                                                 stop=(do == DC - 1))

                            def to_h(c2, t3):
                                hc = p4h.tile([128, M], F32, tag="hc", name="hc")
                                nc.vector.tensor_mul(hc, t3, consts["rowmask"])
                                nc.sync.dma_start(out=hT_r[c2], in_=hc)

                            ln_apply(y2_t, psum_sum2, psum_ssq2, consts["l2g"], consts["l2b"],
                                     to_h)

    nc.compile()
    return nc


def _shared_arrays(inputs):
    f32 = np.float32
    w_qkv = np.ascontiguousarray(inputs["w_qkv"], dtype=f32)
    b_qkv = np.ascontiguousarray(inputs["b_qkv"], dtype=f32)

    def pc(v):  # [C*128] -> [128, C] column-chunk layout
        v = np.ascontiguousarray(v, dtype=f32)
        return np.ascontiguousarray(v.reshape(-1, 128).T)

    w_fc_raw = np.ascontiguousarray(inputs["w_fc"], dtype=np.float64)
    wfcg = (w_fc_raw * np.asarray(inputs["ln1_g"],
                                  dtype=np.float64)[:, None]).astype(f32)
    wv_ext = np.zeros((D, VW), f32)
    bv_ext = np.zeros((1, VW), f32)
    for h in range(H):
        wv_ext[:, h * (DH + 1):h * (DH + 1) + DH] = \
            w_qkv[:, 2 * D + h * DH:2 * D + (h + 1) * DH]
        bv_ext[0, h * (DH + 1):h * (DH + 1) + DH] = \
            b_qkv[2 * D + h * DH:2 * D + (h + 1) * DH]
        bv_ext[0, h * (DH + 1) + DH] = 1.0

    return dict(
        wq=np.ascontiguousarray(w_qkv[:, 0:D]),
        bq_pc=pc(b_qkv[0:D]),
        wk=np.ascontiguousarray(w_qkv[:, D:2 * D]),
        bk_pc=pc(b_qkv[D:2 * D]),
        wv=wv_ext,
        bv=bv_ext,
        wproj=np.ascontiguousarray(inputs["w_proj"], dtype=f32),
        wfc=wfcg,
        wt1=(-wfcg.sum(axis=0, dtype=np.float64)).astype(f32)[None, :],
        wt2=(w_fc_raw * np.asarray(inputs["ln1_b"], dtype=np.float64)[:, None]
             ).sum(axis=0).astype(f32)[None, :],
        wout=np.ascontiguousarray(inputs["w_out"], dtype=f32),
        onesr=np.ones((1, M), f32),
        epsc=np.full((128, 1), EPS, f32),
        ones128=np.ones((128, 128), f32),
        bprj=pc(inputs["b_proj"]),
        bfc=pc(inputs["b_fc"]),
        bout=pc(inputs["b_out"]),
        l1g=pc(inputs["ln1_g"]),
        l1b=pc(inputs["ln1_b"]),
        l2g=pc(inputs["ln2_g"]),
        l2b=pc(inputs["ln2_b"]),
    )


def make_in_maps(inputs):
    inputs = {k: np.asarray(v) for k, v in inputs.items()}
    x = np.ascontiguousarray(inputs["x"], dtype=np.float32)
    lengths = np.asarray(inputs["lengths"]).astype(np.int64)
    shared = _shared_arrays(inputs)
    pos = np.arange(T)
    in_maps = []
    for c in range(8):
        b, r = divmod(c, 4)
        sl = slice(r * M, (r + 1) * M)
        xTb = np.ascontiguousarray(x[b].T)
        km = (pos < lengths[b]).astype(np.float32)
        rm = (pos[sl] < lengths[b]).astype(np.float32)
        m = dict(shared)
        m["xT"] = xTb
        m["xTq"] = np.ascontiguousarray(xTb[:, sl])
        m["vmask"] = np.ascontiguousarray(km.reshape(NKC, 128).T)
        m["rowmask"] = np.ascontiguousarray(np.broadcast_to(rm[None, :], (128, M)))
        in_maps.append(m)
    return in_maps


def get_program(reps=1):
    key = f"nc{reps}"
    if key not in _STATE:
        _STATE[key] = _build_program(reps)
    return _STATE[key]


def kernel(**inputs) -> np.ndarray:
    from concourse.bass_utils import run_bass_kernel_spmd

    nc = get_program()
    in_maps = make_in_maps(inputs)
    res = run_bass_kernel_spmd(nc, in_maps, list(range(8)), trace=False)
    out = np.zeros((B, T, D), np.float32)
    for c in range(8):
        b, r = divmod(c, 4)
        out[b, r * M:(r + 1) * M, :] = res.results[c]["hT"].T
    return out



# revision 18
# speedup vs baseline: 588.4887x; 588.4887x over previous
"""Trainium2 Bass kernel for nn_Block (ragged transformer block).

B=2, T=2048, D=768, H=12, DH=64, FF=3072.

Sharding: 8 cores = 2 batches x 4 query-blocks of 512 tokens.
Each core computes K^T/V over the full sequence of its batch (replicated
within the 4-core batch group), and Q/attention/proj/LN/FFN for its own
512 tokens. No collectives; the host gathers the 8 disjoint output slabs.

On-chip layout is feature-major ("transposed", [feature, token]) end to
end, so every matmul chains with no transposes:
  xT -> Q^T/K^T (feat-major) and V (token-major) -> S^T = K^T.T @ Q^T
  -> exp -> U^T = V'.T @ expS^T (softmax sums via ones-bias columns baked
  into V') -> a^T -> proj -> LN1 (stats via ones-matmul) -> FFN -> LN2.

All matmul operands are bfloat16 (full PE rate, half DMA/SBUF of fp32),
accumulation in fp32 PSUM. Weights are host-repacked so every streamed
chunk is one fat contiguous DMA line per partition. V' bias (incl. the
softmax-denominator ones columns) is written into PSUM by the idle Act
engine instead of PE matmuls. LN1 is applied (t2 = (y1-mu)*rstd, bf16)
before the FFN contraction with ln1_g folded into w_fc and
b_fc + w_fc^T ln1_b folded into the gelu bias. Row masks: masked keys
are zeroed via V' column scaling; padded-query rows carry finite junk
that the host zeroes exactly during unsharding (same as the reference's
final row mask).
"""
import sys
for _p in ("/opt/trn_rl_repo", "/root/.axon_site/_ro/trn_rl_repo"):
    if _p not in sys.path:
        sys.path.append(_p)

from contextlib import ExitStack
import numpy as np

B, T, D, H, DH, FF = 2, 2048, 768, 12, 64, 3072
M = 512            # tokens per core
DC = 6             # D / 128
FC = 24            # FF / 128
NKC = 16           # T / 128
VW = H * (DH + 1)  # 780: V' width (64 cols + 1 ones-bias col per head)
EPS = 1e-5

_STATE: dict = {}


def _build_program(reps=1):
    import concourse.mybir as mybir
    import concourse.tile as tile
    from concourse import bacc

    F32 = mybir.dt.float32
    BF16 = mybir.dt.bfloat16
    AF = mybir.ActivationFunctionType
    OP = mybir.AluOpType

    nc = bacc.Bacc("TRN2", target_bir_lowering=False, debug=False, num_devices=8)

    def din(name, shape, dt=BF16):
        return nc.dram_tensor(name, shape, dt, kind="ExternalInput").ap()

    xT = din("xT", [D, T])
    xTq = din("xTq", [D, M])
    # weights host-repacked: [128, out_chunk, in_chunk, 128] so the DMA for
    # one out-chunk is a single contiguous line per partition
    wq = din("wq", [128, DC, DC, 128])
    wk = din("wk", [128, DC, DC, 128])
    bq_pc = din("bq_pc", [128, DC], F32)
    bk_pc = din("bk_pc", [128, DC], F32)
    wv = din("wv", [D, VW])
    bvb = din("bvb", [128, VW])
    wproj = din("wproj", [D, D])
    wfc = din("wfc", [128, FC, DC, 128])
    wout = din("wout", [FF, D])
    ones128 = din("ones128", [128, 128])
    bprj = din("bprj", [128, DC], F32)
    bfc = din("bfc", [128, FC], F32)
    bout = din("bout", [128, DC], F32)
    l1g = din("l1g", [128, DC], F32)
    l1b = din("l1b", [128, DC], F32)
    l2g = din("l2g", [128, DC], F32)
    l2b = din("l2b", [128, DC], F32)
    vmask = din("vmask", [128, NKC], F32)
    epsc = din("epsc", [128, 1], F32)

    hT = nc.dram_tensor("hT", [D, M], F32, kind="ExternalOutput").ap()

    xT_r = xT.rearrange("(c p) n -> p c n", p=128)
    xTq_r = xTq.rearrange("(c p) n -> p c n", p=128)
    wv_r = wv.rearrange("(c p) n -> p c n", p=128)
    wproj_r = wproj.rearrange("(c p) n -> p c n", p=128)
    hT_r = hT.rearrange("(c p) n -> c p n", p=128)

    # exp group sizes per head: 16 key-chunks in groups of 3 (PSUM fits
    # 2x3-bank s-tiles + 2x1-bank u-tiles)
    EG = [3, 3, 3, 3, 2, 2]

    with tile.TileContext(nc) as tc, ExitStack() as ctx:
        ctx.enter_context(nc.allow_low_precision(
            reason="bf16 pipeline; output tolerance 2e-2"))
        const = ctx.enter_context(tc.tile_pool(name="const", bufs=1))
        # persistent per-rep tiles (xTq stays resident through phase 3);
        # bufs=2 so the next rep's loads overlap this rep's tail
        persist = ctx.enter_context(tc.tile_pool(name="persist", bufs=2))
        # weight-streaming pools at top level: fresh SBUF regions, so their
        # DMAs never WAR-wait on attention-phase tiles and prefetch freely
        wpre = ctx.enter_context(tc.tile_pool(name="wpre", bufs=1))
        p4w = ctx.enter_context(tc.tile_pool(name="p4w", bufs=4))

        def cload(name, shape, dt, src):
            t = const.tile(shape, dt, tag=name, name=name + "_t")
            nc.sync.dma_start(out=t, in_=src)
            return t

        consts = {}

        def cloads():
            consts["ones128"] = cload("ones128", [128, 128], BF16, ones128)
            consts["bvb"] = cload("bvb", [128, VW], BF16, bvb)
            consts["vmask"] = cload("vmask", [128, NKC], F32, vmask)
            consts["epsc"] = cload("epsc", [128, 1], F32, epsc)
            consts["bprj"] = cload("bprj", [128, DC], F32, bprj)
            consts["bfc"] = cload("bfc", [128, FC], F32, bfc)
            consts["bout"] = cload("bout", [128, DC], F32, bout)
            consts["l1g"] = cload("l1g", [128, DC], F32, l1g)
            consts["l1b"] = cload("l1b", [128, DC], F32, l1b)
            consts["l2g"] = cload("l2g", [128, DC], F32, l2g)
            consts["l2b"] = cload("l2b", [128, DC], F32, l2b)

        # ---------------- LN helpers (transposed layout) ----------------
        def ln_stats_bc(lnp_t, lnp_k, psum_sum, psum_ssq):
            m_bc = lnp_k.tile([128, M], BF16, tag="mbc", name="mbc")
            nc.vector.tensor_scalar_mul(m_bc, psum_sum, 1.0 / D)
            mm = lnp_t.tile([128, M], F32, tag="mm", name="mm")
            nc.vector.tensor_mul(mm, m_bc, m_bc)
            var = lnp_t.tile([128, M], F32, tag="var", name="var")
            nc.vector.scalar_tensor_tensor(var, psum_ssq, 1.0 / D, mm,
                                           op0=OP.mult, op1=OP.subtract)
            sd = lnp_t.tile([128, M], F32, tag="sd", name="sd")
            nc.scalar.activation(sd, var, AF.Sqrt, bias=consts["epsc"][:, 0:1])
            rstd = lnp_k.tile([128, M], BF16, tag="rstd", name="rstd")
            nc.vector.reciprocal(rstd, sd)
            return m_bc, rstd

        big = ctx.enter_context(tc.tile_pool(name="big", bufs=1))
        for _rep in range(reps):
            kT_t = big.tile([128, DC, T], BF16, tag="slotL", name="kT")
            vP_t = big.tile([128, NKC, VW], BF16, tag="slotM", name="vP")
            qT_t = big.tile([128, DC, M], BF16, tag="slotS", name="qT")
            xTq_t = persist.tile([128, DC, M], BF16, tag="xTq", name="xTq_t")
            for dc in range(DC):
                nc.gpsimd.dma_start(out=xTq_t[:, dc, :], in_=xTq_r[:, dc, :])

            # -------- Phase 1: QKV projections --------
            # all non-FFN weights are small in bf16 (~4.7MB total): make
            # them fully resident up front so nothing downstream waits on
            # just-in-time weight chunks
            wq_t = wpre.tile([128, DC, DC, 128], BF16, tag="wq", name="wq_t")
            for qc in range(DC):
                nc.sync.dma_start(out=wq_t[:, qc], in_=wq[:, qc])
            bq_t = cload("bq_pc", [128, DC], F32, bq_pc)
            bk_t = cload("bk_pc", [128, DC], F32, bk_pc)
            wk_t = wpre.tile([128, DC, DC, 128], BF16, tag="wk", name="wk_t")
            for kc in range(DC):
                nc.sync.dma_start(out=wk_t[:, kc], in_=wk[:, kc])
            cloads()
            wv_t = wpre.tile([128, DC, VW], BF16, tag="wv", name="wv_t")
            nc.sync.dma_start(out=wv_t, in_=wv_r)
            wproj_t = wpre.tile([128, DC, D], BF16, tag="wproj",
                                name="wproj_t")
            nc.sync.dma_start(out=wproj_t, in_=wproj_r)
            with tc.tile_pool(name="p1x", bufs=2) as p1x:
                # Q^T [768, 512]
                with tc.tile_pool(name="psQ", bufs=2, space="PSUM") as psQ:
                    for qc in range(DC):
                        pq = psQ.tile([128, M], F32, tag="pq", name="pq")
                        for dc in range(DC):
                            nc.tensor.matmul(pq, wq_t[:, qc, dc, :],
                                             xTq_t[:, dc, :],
                                             start=(dc == 0), stop=(dc == DC - 1))
                        nc.vector.tensor_scalar_add(qT_t[:, qc, :], pq,
                                                    bq_t[:, qc:qc + 1])

                # K^T / V' per token-block
                with tc.tile_pool(name="psK", bufs=4, space="PSUM") as psK, \
                     tc.tile_pool(name="psV", bufs=4, space="PSUM") as psV:
                    for tb in range(4):
                        xtb = p1x.tile([128, DC, M], BF16, tag="xtb",
                                       name="xtb")
                        for dc in range(DC):
                            nc.gpsimd.dma_start(
                                out=xtb[:, dc, :],
                                in_=xT_r[:, dc, tb * M:(tb + 1) * M])
                        for kc in range(DC):
                            pk = psK.tile([128, M], F32, tag="pk", name="pk")
                            for dc in range(DC):
                                nc.tensor.matmul(pk, wk_t[:, kc, dc, :],
                                                 xtb[:, dc, :],
                                                 start=(dc == 0),
                                                 stop=(dc == DC - 1))
                            nc.vector.tensor_scalar_add(
                                kT_t[:, kc, tb * M:(tb + 1) * M], pk,
                                bk_t[:, kc:kc + 1])
                        for tq in range(4):
                            tci = tb * 4 + tq
                            for vb in range(2):
                                pv = psV.tile([128, VW // 2], F32, tag="pv",
                                              name="pv")
                                # V' bias (incl. softmax ones cols) via Act
                                nc.scalar.activation(
                                    pv,
                                    consts["bvb"][:,
                                                  vb * (VW // 2):(vb + 1) * (VW // 2)],
                                    AF.Copy)
                                for dc in range(DC):
                                    nc.tensor.matmul(
                                        pv, xtb[:, dc, tq * 128:(tq + 1) * 128],
                                        wv_t[:, dc,
                                             vb * (VW // 2):(vb + 1) * (VW // 2)],
                                        start=False, stop=(dc == DC - 1),
                                        skip_group_check=True)
                                nc.vector.tensor_scalar_mul(
                                    vP_t[:, tci,
                                         vb * (VW // 2):(vb + 1) * (VW // 2)],
                                    pv, consts["vmask"][:, tci:tci + 1])

            # -------- Phase 2: attention (exp in groups of 3) --------
            if True:
                aT_t = big.tile([128, DC, M], BF16, tag="slotA", name="aT")
                with tc.tile_pool(name="attp", bufs=3) as attp, \
                     tc.tile_pool(name="atts", bufs=2) as atts, \
                     tc.tile_pool(name="psS", bufs=2, space="PSUM") as psS, \
                     tc.tile_pool(name="psU", bufs=2, space="PSUM") as psU:
                    for h in range(H):
                        po = (h % 2) * 64
                        chk = h // 2
                        pu = psU.tile([128, M], F32, tag="pu", name="pu")
                        kc = 0
                        for g in EG:
                            s3 = psS.tile([128, 3, M], F32, tag="s", name="s3")
                            for j in range(g):
                                nc.tensor.matmul(
                                    s3[:, j, :],
                                    kT_t[po:po + 64, chk,
                                         (kc + j) * 128:(kc + j + 1) * 128],
                                    qT_t[po:po + 64, chk, :],
                                    start=True, stop=True)
                            e3 = attp.tile([128, 3, M], BF16, tag="exp",
                                           name="e3")
                            nc.scalar.activation(e3[:, 0:g, :], s3[:, 0:g, :],
                                                 AF.Exp)
                            for j in range(g):
                                nc.tensor.matmul(
                                    pu[0:DH + 1, :],
                                    vP_t[:, kc + j,
                                         h * (DH + 1):(h + 1) * (DH + 1)],
                                    e3[:, j, :],
                                    start=(kc + j == 0),
                                    stop=(kc + j == NKC - 1))
                            kc += g
                        srow = atts.tile([1, M], F32, tag="srow", name="srow")
                        nc.vector.tensor_copy(srow, pu[DH:DH + 1, :])
                        sbc = atts.tile([64, M], F32, tag="sbc", name="sbc")
                        nc.gpsimd.partition_broadcast(sbc, srow, channels=64)
                        rinv = atts.tile([64, M], F32, tag="rinv", name="rinv")
                        nc.vector.reciprocal(rinv, sbc)
                        nc.vector.tensor_mul(aT_t[po:po + 64, chk, :],
                                             pu[0:DH, :], rinv)

            # ------------ Phase 3: proj + residual + LN1 ------------
            with tc.tile_pool(name="foldp", bufs=1) as foldp:
                    nT_t = big.tile([128, DC, M], BF16, tag="slotS", name="nT")
                    t1T = big.tile([128, DC, M], BF16, tag="slotB", name="t1T")
                    with tc.tile_pool(name="p3", bufs=1) as p3, \
                         tc.tile_pool(name="p3s", bufs=2) as p3s, \
                         tc.tile_pool(name="psP", bufs=2, space="PSUM") as psP, \
                         tc.tile_pool(name="psT", bufs=1, space="PSUM") as psT:
                        y1_t = big.tile([128, DC, M], BF16, tag="slotL",
                                        name="y1")
                        psum_sum = psT.tile([128, M], F32, tag="s1",
                                            name="psum_sum")
                        psum_ssq = psT.tile([128, M], F32, tag="s2",
                                            name="psum_ssq")
                        for do in range(DC):
                            pp = psP.tile([128, M], F32, tag="pp", name="pp")
                            for di in range(DC):
                                nc.tensor.matmul(
                                    pp, wproj_t[:, di, do * 128:(do + 1) * 128],
                                    aT_t[:, di, :], start=(di == 0),
                                    stop=(di == DC - 1))
                            nc.vector.scalar_tensor_tensor(
                                y1_t[:, do, :], pp, consts["bprj"][:, do:do + 1],
                                xTq_t[:, do, :], op0=OP.add, op1=OP.add)
                            sq = p3s.tile([128, M], BF16, tag="sq", name="sq")
                            nc.vector.tensor_mul(sq, y1_t[:, do, :],
                                                 y1_t[:, do, :])
                            nc.tensor.matmul(psum_sum, consts["ones128"],
                                             y1_t[:, do, :],
                                             start=(do == 0), stop=(do == DC - 1))
                            nc.tensor.matmul(psum_ssq, consts["ones128"], sq,
                                             start=(do == 0), stop=(do == DC - 1))

                        # t1 = y1 - mu (bf16): the FFN contraction input.
                        # rstd is folded in per-f on the PSUM result, so the
                        # sqrt/reciprocal chain is off the FFN1 start path.
                        m_bc, rstd_bc = ln_stats_bc(p3, foldp,
                                                    psum_sum, psum_ssq)
                        with tc.tile_pool(name="lnx", bufs=2) as lnx:
                            for c2 in range(DC):
                                nc.vector.tensor_sub(t1T[:, c2, :],
                                                     y1_t[:, c2, :], m_bc)

                    # ------------ Phase 4: FFN + residual + LN2 ------------
                    with tc.tile_pool(name="p4a", bufs=2) as p4a, \
                         tc.tile_pool(name="p4h", bufs=2) as p4h, \
                         tc.tile_pool(name="psM", bufs=1, space="PSUM") as psM:
                        psm = [psM.tile([128, M], F32, tag=f"m{do}",
                                        name=f"psm{do}") for do in range(DC)]
                        with tc.tile_pool(name="lnx2", bufs=2) as lnx2, \
                             tc.tile_pool(name="psF", bufs=2,
                                          space="PSUM") as psF:
                            for f in range(FC):
                                wfcf = p4w.tile([128, DC, 128], BF16, tag="wfcf",
                                                name="wfcf")
                                nc.sync.dma_start(out=wfcf, in_=wfc[:, f])
                                woutf = p4w.tile([128, D], BF16, tag="woutf",
                                                 name="woutf")
                                nc.sync.dma_start(
                                    out=woutf, in_=wout[f * 128:(f + 1) * 128, :])
                                pf = psF.tile([128, M], F32, tag="pf", name="pf")
                                for dc in range(DC):
                                    nc.tensor.matmul(pf, wfcf[:, dc, :],
                                                     t1T[:, dc, :],
                                                     start=(dc == 0),
                                                     stop=(dc == DC - 1))
                                nc.vector.tensor_mul(pf, pf, rstd_bc)
                                a1 = p4a.tile([128, M], BF16, tag="a1", name="a1")
                                nc.scalar.activation(a1, pf, AF.Gelu_apprx_tanh,
                                                     bias=consts["bfc"][:, f:f + 1])
                                for do in range(DC):
                                    nc.tensor.matmul(
                                        psm[do],
                                        woutf[:, do * 128:(do + 1) * 128],
                                        a1, start=(f == 0), stop=(f == FC - 1))
                                # nT = (t1*rstd)*g + b, interleaved so DVE
                                # stays off the pf critical path
                                if f % 4 == 2 and f // 4 < DC:
                                    c2 = f // 4
                                    t2 = lnx2.tile([128, M], BF16, tag="t2",
                                                   name="t2")
                                    nc.vector.tensor_mul(t2, t1T[:, c2, :],
                                                         rstd_bc)
                                    nc.vector.tensor_scalar(
                                        nT_t[:, c2, :], t2,
                                        consts["l1g"][:, c2:c2 + 1],
                                        consts["l1b"][:, c2:c2 + 1],
                                        op0=OP.mult, op1=OP.add)

                        with tc.tile_pool(name="psT2", bufs=1,
                                          space="PSUM") as psT2:
                            y2_t = big.tile([128, DC, M], BF16,
                                            tag="slotM", name="y2")
                            psum_sum2 = psT2.tile([128, M], F32, tag="s1",
                                                  name="psum_sum2")
                            psum_ssq2 = psT2.tile([128, M], F32, tag="s2",
                                                  name="psum_ssq2")
                            for do in range(DC):
                                nc.vector.scalar_tensor_tensor(
                                    y2_t[:, do, :], psm[do],
                                    consts["bout"][:, do:do + 1],
                                    nT_t[:, do, :],
                                    op0=OP.add, op1=OP.add)
                                sq = p4a.tile([128, M], BF16, tag="sq2",
                                              name="sq2")
                                nc.vector.tensor_mul(
                                    sq, y2_t[:, do, :], y2_t[:, do, :])
                                nc.tensor.matmul(psum_sum2, consts["ones128"],
                                                 y2_t[:, do, :],
                                                 start=(do == 0),
                                                 stop=(do == DC - 1))
                                nc.tensor.matmul(psum_ssq2, consts["ones128"], sq,
                                                 start=(do == 0),
                                                 stop=(do == DC - 1))

                            m2_bc, rstd2_bc = ln_stats_bc(p4h, p4h,
                                                          psum_sum2, psum_ssq2)
                            # padded-query rows are NOT zeroed here; the host
                            # zeroes them exactly during unsharding
                            with tc.tile_pool(name="lnz", bufs=2) as lnz:
                                for c2 in range(DC):
                                    u1 = lnz.tile([128, M], BF16, tag="u1",
                                                  name="u1")
                                    nc.vector.tensor_sub(u1, y2_t[:, c2, :],
                                                         m2_bc)
                                    u2 = lnz.tile([128, M], BF16, tag="u2",
                                                  name="u2")
                                    nc.vector.tensor_mul(u2, u1, rstd2_bc)
                                    hc = lnz.tile([128, M], F32, tag="hc",
                                                  name="hc")
                                    nc.vector.tensor_scalar(
                                        hc, u2, consts["l2g"][:, c2:c2 + 1],
                                        consts["l2b"][:, c2:c2 + 1],
                                        op0=OP.mult, op1=OP.add)
                                    nc.sync.dma_start(out=hT_r[c2], in_=hc)

    nc.compile()
    return nc


def _to_bf16(a):
    import ml_dtypes
    return np.asarray(a, dtype=np.float32).astype(ml_dtypes.bfloat16)


def _pack_w(w, n_out):
    # [D_in, N_out] -> [128, N_out/128, D_in/128, 128]: one contiguous line
    # per partition per out-chunk
    d_in = w.shape[0]
    return np.ascontiguousarray(
        w.reshape(d_in // 128, 128, n_out // 128, 128).transpose(1, 2, 0, 3))


def _shared_arrays(inputs):
    f32 = np.float32
    w_qkv = np.ascontiguousarray(inputs["w_qkv"], dtype=f32)
    b_qkv = np.ascontiguousarray(inputs["b_qkv"], dtype=f32)

    def pc(v):  # [C*128] -> [128, C] column-chunk layout
        v = np.ascontiguousarray(v, dtype=f32)
        return np.ascontiguousarray(v.reshape(-1, 128).T)

    w_fc_raw = np.ascontiguousarray(inputs["w_fc"], dtype=np.float64)
    ln1_g = np.asarray(inputs["ln1_g"], dtype=np.float64)
    ln1_b = np.asarray(inputs["ln1_b"], dtype=np.float64)
    wfcg = (w_fc_raw * ln1_g[:, None]).astype(f32)
    # gelu bias: b_fc + w_fc^T ln1_b (the +b part of LN1 folded out of the
    # FFN contraction)
    cb = (np.asarray(inputs["b_fc"], dtype=np.float64)
          + (w_fc_raw * ln1_b[:, None]).sum(axis=0)).astype(f32)
    wv_ext = np.zeros((D, VW), f32)
    bv_ext = np.zeros((VW,), f32)
    for h in range(H):
        wv_ext[:, h * (DH + 1):h * (DH + 1) + DH] = \
            w_qkv[:, 2 * D + h * DH:2 * D + (h + 1) * DH]
        bv_ext[h * (DH + 1):h * (DH + 1) + DH] = \
            b_qkv[2 * D + h * DH:2 * D + (h + 1) * DH]
        bv_ext[h * (DH + 1) + DH] = 1.0

    return dict(
        wq=_to_bf16(_pack_w(w_qkv[:, 0:D], D)),
        bq_pc=pc(b_qkv[0:D]),
        wk=_to_bf16(_pack_w(w_qkv[:, D:2 * D], D)),
        bk_pc=pc(b_qkv[D:2 * D]),
        wv=_to_bf16(wv_ext),
        bvb=_to_bf16(np.broadcast_to(bv_ext[None, :], (128, VW))),
        wproj=_to_bf16(np.asarray(inputs["w_proj"], dtype=f32)),
        wfc=_to_bf16(_pack_w(wfcg, FF)),
        wout=_to_bf16(np.asarray(inputs["w_out"], dtype=f32)),
        epsc=np.full((128, 1), EPS, f32),
        ones128=_to_bf16(np.ones((128, 128), f32)),
        bprj=pc(inputs["b_proj"]),
        bfc=pc(cb),
        bout=pc(inputs["b_out"]),
        l1g=pc(inputs["ln1_g"]),
        l1b=pc(inputs["ln1_b"]),
        l2g=pc(inputs["ln2_g"]),
        l2b=pc(inputs["ln2_b"]),
    )


def make_in_maps(inputs):
    inputs = {k: np.asarray(v) for k, v in inputs.items()}
    x = np.ascontiguousarray(inputs["x"], dtype=np.float32)
    lengths = np.asarray(inputs["lengths"]).astype(np.int64)
    shared = _shared_arrays(inputs)
    pos = np.arange(T)
    in_maps = []
    for c in range(8):
        b, r = divmod(c, 4)
        sl = slice(r * M, (r + 1) * M)
        xTb = _to_bf16(x[b].T)
        km = (pos < lengths[b]).astype(np.float32)
        m = dict(shared)
        m["xT"] = xTb
        m["xTq"] = np.ascontiguousarray(xTb[:, sl])
        m["vmask"] = np.ascontiguousarray(km.reshape(NKC, 128).T)
        in_maps.append(m)
    return in_maps


def get_program(reps=1):
    key = f"nc{reps}"
    if key not in _STATE:
        _STATE[key] = _build_program(reps)
    return _STATE[key]


def kernel(**inputs) -> np.ndarray:
    from concourse.bass_utils import run_bass_kernel_spmd

    nc = get_program()
    in_maps = make_in_maps(inputs)
    res = run_bass_kernel_spmd(nc, in_maps, list(range(8)), trace=False)
    out = np.zeros((B, T, D), np.float32)
    for c in range(8):
        b, r = divmod(c, 4)
        out[b, r * M:(r + 1) * M, :] = res.results[c]["hT"].T
    # zero padded-query rows exactly (the reference's final row mask)
    lengths = np.asarray(inputs["lengths"]).astype(np.int64)
    for b in range(B):
        out[b, lengths[b]:, :] = 0.0
    return out


# revision 45
# speedup vs baseline: 592.7882x; 1.0073x over previous
"""Trainium2 Bass kernel for nn_Block (ragged transformer block).

B=2, T=2048, D=768, H=12, DH=64, FF=3072.

Sharding: 8 cores = 2 batches x 4 query-blocks of 512 tokens.
Each core computes K^T/V over the full sequence of its batch (replicated
within the 4-core batch group), and Q/attention/proj/LN/FFN for its own
512 tokens. No collectives; the host gathers the 8 disjoint output slabs.

On-chip layout is feature-major ("transposed", [feature, token]) end to
end, so every matmul chains with no transposes:
  xT -> Q^T/K^T (feat-major) and V (token-major) -> S^T = K^T.T @ Q^T
  -> exp -> U^T = V'.T @ expS^T (softmax sums via ones-bias columns baked
  into V') -> a^T -> proj -> LN1 (stats via ones-matmul) -> FFN -> LN2.

All matmul operands are bfloat16 (full PE rate, half the DMA/SBUF of
fp32), accumulation in fp32 PSUM; fp32 is kept on the output-side path
(LN stats, rstd, nT, y2 drains) to hold rel err ~4e-3. Weights are
host-repacked so every streamed chunk is one fat contiguous DMA line
per partition; all non-FFN weights are made fully resident up front and
FFN weights double-buffered 6 deep across two DMA queues. V' bias
(incl. the softmax-denominator ones columns) is folded into the DVE
PSUM drain (vP = pv*vmask + bv*vmask). The FFN consumes mean-centered
t1 = y1-mu with rstd applied per-f on the PSUM result, keeping the
sqrt/reciprocal chain off the FFN1 start path; ln1_g is folded into
w_fc and b_fc + w_fc^T ln1_b into the gelu bias. exp runs in uniform
trios over the flat (head, key-chunk) list (softmax is permutation
invariant), which keeps the Act engine's fixed per-instruction overhead
minimal and the cadence smooth. Row masks: masked keys are zeroed via
the V' drain; padded-query rows carry finite junk that the host zeroes
exactly during unsharding (same as the reference's final row mask).
"""
import sys
for _p in ("/opt/trn_rl_repo", "/root/.axon_site/_ro/trn_rl_repo"):
    if _p not in sys.path:
        sys.path.append(_p)

from contextlib import ExitStack
import numpy as np

B, T, D, H, DH, FF = 2, 2048, 768, 12, 64, 3072
M = 512            # tokens per core
DC = 6             # D / 128
FC = 24            # FF / 128
NKC = 16           # T / 128
VW = H * (DH + 1)  # 780: V' width (64 cols + 1 ones-bias col per head)
EPS = 1e-5

_STATE: dict = {}


def _build_program(reps=1):
    import concourse.mybir as mybir
    import concourse.tile as tile
    from concourse import bacc

    F32 = mybir.dt.float32
    F32R = mybir.dt.float32r
    BF16 = mybir.dt.bfloat16
    AF = mybir.ActivationFunctionType
    OP = mybir.AluOpType

    nc = bacc.Bacc("TRN2", target_bir_lowering=False, debug=False, num_devices=8)

    def din(name, shape, dt=BF16):
        return nc.dram_tensor(name, shape, dt, kind="ExternalInput").ap()

    xT = din("xT", [D, T])
    xTq = din("xTq", [D, M])
    # weights host-repacked: [128, out_chunk, in_chunk, 128] so the DMA for
    # one out-chunk is a single contiguous line per partition
    wq = din("wq", [128, DC, DC, 128])
    wk = din("wk", [128, DC, DC, 128])
    bq_pc = din("bq_pc", [128, DC], F32)
    bk_pc = din("bk_pc", [128, DC], F32)
    wv = din("wv", [D, VW])
    bvb = din("bvb", [128, VW])
    wproj = din("wproj", [D, D])
    wfc = din("wfc", [128, FC, DC, 128])
    wout = din("wout", [FF, D])
    ones128 = din("ones128", [128, 128])
    bprj = din("bprj", [128, DC], F32)
    bfc = din("bfc", [128, FC], F32)
    bout = din("bout", [128, DC], F32)
    l1g = din("l1g", [128, DC], F32)
    l1b = din("l1b", [128, DC], F32)
    l2g = din("l2g", [128, DC], F32)
    l2b = din("l2b", [128, DC], F32)
    vmask = din("vmask", [128, NKC], F32)
    epsc = din("epsc", [128, 1], F32)

    hT = nc.dram_tensor("hT", [D, M], F32, kind="ExternalOutput").ap()

    xT_r = xT.rearrange("(c p) n -> p c n", p=128)
    xTq_r = xTq.rearrange("(c p) n -> p c n", p=128)
    wv_r = wv.rearrange("(c p) n -> p c n", p=128)
    wproj_r = wproj.rearrange("(c p) n -> p c n", p=128)
    hT_r = hT.rearrange("(c p) n -> c p n", p=128)

    with tile.TileContext(nc) as tc, ExitStack() as ctx:
        ctx.enter_context(nc.allow_low_precision(
            reason="bf16 pipeline; output tolerance 2e-2"))
        const = ctx.enter_context(tc.tile_pool(name="const", bufs=1))
        # persistent per-rep tiles (xTq stays resident through phase 3);
        # bufs=2 so the next rep's loads overlap this rep's tail
        persist = ctx.enter_context(tc.tile_pool(name="persist", bufs=2))
        # weight-streaming pools at top level: fresh SBUF regions, so their
        # DMAs never WAR-wait on attention-phase tiles and prefetch freely
        wpre = ctx.enter_context(tc.tile_pool(name="wpre", bufs=1))
        p4w = ctx.enter_context(tc.tile_pool(name="p4w", bufs=6))

        def cload(name, shape, dt, src):
            t = const.tile(shape, dt, tag=name, name=name + "_t")
            nc.sync.dma_start(out=t, in_=src)
            return t

        consts = {}

        def cloads():
            consts["ones128"] = cload("ones128", [128, 128], BF16, ones128)
            consts["bvb"] = cload("bvb", [128, VW], BF16, bvb)
            consts["vmask"] = cload("vmask", [128, NKC], F32, vmask)
            consts["epsc"] = cload("epsc", [128, 1], F32, epsc)
            consts["bprj"] = cload("bprj", [128, DC], F32, bprj)
            consts["bfc"] = cload("bfc", [128, FC], F32, bfc)
            consts["bout"] = cload("bout", [128, DC], F32, bout)
            consts["l1g"] = cload("l1g", [128, DC], F32, l1g)
            consts["l1b"] = cload("l1b", [128, DC], F32, l1b)
            consts["l2g"] = cload("l2g", [128, DC], F32, l2g)
            consts["l2b"] = cload("l2b", [128, DC], F32, l2b)

        # ---------------- LN helpers (transposed layout) ----------------
        def ln_stats_bc(lnp_t, lnp_k, psum_sum, psum_ssq, sdt=None):
            sdt = sdt or F32
            m_bc = lnp_k.tile([128, M], sdt, tag="mbc", name="mbc")
            nc.vector.tensor_scalar_mul(m_bc, psum_sum, 1.0 / D)
            mm = lnp_t.tile([128, M], F32, tag="mm", name="mm")
            nc.vector.tensor_mul(mm, m_bc, m_bc)
            var = lnp_t.tile([128, M], F32, tag="var", name="var")
            nc.vector.scalar_tensor_tensor(var, psum_ssq, 1.0 / D, mm,
                                           op0=OP.mult, op1=OP.subtract)
            sd = lnp_t.tile([128, M], F32, tag="sd", name="sd")
            nc.scalar.activation(sd, var, AF.Sqrt, bias=consts["epsc"][:, 0:1])
            rstd = lnp_k.tile([128, M], sdt, tag="rstd", name="rstd")
            nc.vector.reciprocal(rstd, sd)
            return m_bc, rstd

        big = ctx.enter_context(tc.tile_pool(name="big", bufs=1))
        for _rep in range(reps):
            kT_t = big.tile([128, DC, T], BF16, tag="slotL", name="kT")
            vP_t = big.tile([128, NKC, VW], BF16, tag="slotM", name="vP")
            qT_t = big.tile([128, DC, M], BF16, tag="slotS", name="qT")
            xTq_t = persist.tile([128, DC, M], BF16, tag="xTq", name="xTq_t")
            for dc in range(DC):
                nc.gpsimd.dma_start(out=xTq_t[:, dc, :], in_=xTq_r[:, dc, :])

            # -------- Phase 1: QKV projections --------
            # all non-FFN weights are small in bf16 (~4.7MB total): make
            # them fully resident up front so nothing downstream waits on
            # just-in-time weight chunks
            wq_t = wpre.tile([128, DC, DC, 128], BF16, tag="wq", name="wq_t")
            for qc in range(DC):
                nc.sync.dma_start(out=wq_t[:, qc], in_=wq[:, qc])
            bq_t = cload("bq_pc", [128, DC], F32, bq_pc)
            bk_t = cload("bk_pc", [128, DC], F32, bk_pc)
            wk_t = wpre.tile([128, DC, DC, 128], BF16, tag="wk", name="wk_t")
            for kc in range(DC):
                nc.sync.dma_start(out=wk_t[:, kc], in_=wk[:, kc])
            cloads()
            wv_t = wpre.tile([128, DC, VW], BF16, tag="wv", name="wv_t")
            nc.sync.dma_start(out=wv_t, in_=wv_r)
            wproj_t = wpre.tile([128, DC, D], BF16, tag="wproj",
                                name="wproj_t")
            nc.sync.dma_start(out=wproj_t, in_=wproj_r)
            with tc.tile_pool(name="p1x", bufs=2) as p1x:
                # Q^T [768, 512]
                with tc.tile_pool(name="psQ", bufs=2, space="PSUM") as psQ:
                    for qc in range(DC):
                        pq = psQ.tile([128, M], F32, tag="pq", name="pq")
                        for dc in range(DC):
                            nc.tensor.matmul(pq, wq_t[:, qc, dc, :],
                                             xTq_t[:, dc, :],
                                             start=(dc == 0), stop=(dc == DC - 1))
                        nc.vector.tensor_scalar_add(qT_t[:, qc, :], pq,
                                                    bq_t[:, qc:qc + 1])

                # K^T / V' per token-block
                with tc.tile_pool(name="psK", bufs=4, space="PSUM") as psK, \
                     tc.tile_pool(name="psV", bufs=4, space="PSUM") as psV:
                    for tb in range(4):
                        xtb = p1x.tile([128, DC, M], BF16, tag="xtb",
                                       name="xtb")
                        for dc in range(DC):
                            nc.gpsimd.dma_start(
                                out=xtb[:, dc, :],
                                in_=xT_r[:, dc, tb * M:(tb + 1) * M])
                        for kc in range(DC):
                            pk = psK.tile([128, M], F32, tag="pk", name="pk")
                            for dc in range(DC):
                                nc.tensor.matmul(pk, wk_t[:, kc, dc, :],
                                                 xtb[:, dc, :],
                                                 start=(dc == 0),
                                                 stop=(dc == DC - 1))
                            nc.vector.tensor_scalar_add(
                                kT_t[:, kc, tb * M:(tb + 1) * M], pk,
                                bk_t[:, kc:kc + 1])
                        for tq in range(4):
                            tci = tb * 4 + tq
                            for vb in range(2):
                                pv = psV.tile([128, VW // 2], F32, tag="pv",
                                              name="pv")
                                for dc in range(DC):
                                    nc.tensor.matmul(
                                        pv, xtb[:, dc, tq * 128:(tq + 1) * 128],
                                        wv_t[:, dc,
                                             vb * (VW // 2):(vb + 1) * (VW // 2)],
                                        start=(dc == 0), stop=(dc == DC - 1))
                                # vP = (pv + bv)*vmask = pv*vmask + bvm,
                                # bvm = bv*vmask (incl. softmax ones cols)
                                bvm = p1x.tile([128, VW // 2], BF16, tag="bvm",
                                               name="bvm")
                                nc.vector.tensor_scalar_mul(
                                    bvm,
                                    consts["bvb"][:,
                                                  vb * (VW // 2):(vb + 1) * (VW // 2)],
                                    consts["vmask"][:, tci:tci + 1])
                                nc.vector.scalar_tensor_tensor(
                                    vP_t[:, tci,
                                         vb * (VW // 2):(vb + 1) * (VW // 2)],
                                    pv, consts["vmask"][:, tci:tci + 1], bvm,
                                    op0=OP.mult, op1=OP.add)

            # -------- Phase 2: attention (exp in groups of 3) --------
            if True:
                aT_t = big.tile([128, DC, M], BF16, tag="slotA", name="aT")
                with tc.tile_pool(name="attp", bufs=3) as attp, \
                     tc.tile_pool(name="atts", bufs=2) as atts, \
                     tc.tile_pool(name="psS", bufs=2, space="PSUM") as psS, \
                     tc.tile_pool(name="psU", bufs=2, space="PSUM") as psU:
                    # flat (head, key-chunk) list in uniform trios: softmax
                    # is permutation-invariant over chunks, so groups may
                    # span head boundaries (both heads' pu tiles coexist via
                    # psU bufs=2)
                    flat = [(h, kc) for h in range(H) for kc in range(NKC)]
                    pus = {}

                    def drain_head(h, pu):
                        po = (h % 2) * 64
                        chk = h // 2
                        srow = atts.tile([1, M], F32, tag="srow", name="srow")
                        nc.vector.tensor_copy(srow, pu[DH:DH + 1, :])
                        sbc = atts.tile([64, M], F32, tag="sbc", name="sbc")
                        nc.gpsimd.partition_broadcast(sbc, srow, channels=64)
                        rinv = atts.tile([64, M], F32, tag="rinv", name="rinv")
                        nc.vector.reciprocal(rinv, sbc)
                        nc.vector.tensor_mul(aT_t[po:po + 64, chk, :],
                                             pu[0:DH, :], rinv)

                    for gi in range(0, H * NKC, 3):
                        grp = flat[gi:gi + 3]
                        s3 = psS.tile([128, 3, M], F32, tag="s", name="s3")
                        for j, (h, kc) in enumerate(grp):
                            po = (h % 2) * 64
                            chk = h // 2
                            nc.tensor.matmul(
                                s3[:, j, :],
                                kT_t[po:po + 64, chk,
                                     kc * 128:(kc + 1) * 128],
                                qT_t[po:po + 64, chk, :],
                                start=True, stop=True)
                        e3 = attp.tile([128, 3, M], BF16, tag="exp",
                                       name="e3")
                        nc.scalar.activation(e3, s3, AF.Exp)
                        for j, (h, kc) in enumerate(grp):
                            if kc == 0:
                                pus[h] = psU.tile([128, M], F32, tag="pu",
                                                  name="pu")
                            nc.tensor.matmul(
                                pus[h][0:DH + 1, :],
                                vP_t[:, kc, h * (DH + 1):(h + 1) * (DH + 1)],
                                e3[:, j, :],
                                start=(kc == 0), stop=(kc == NKC - 1))
                            if kc == NKC - 1:
                                drain_head(h, pus.pop(h))

            # ------------ Phase 3: proj + residual + LN1 ------------
            with tc.tile_pool(name="foldp", bufs=1) as foldp:
                    nT_t = big.tile([128, DC, M], F32, tag="slotN", name="nT")
                    t1T = big.tile([128, DC, M], BF16, tag="slotB", name="t1T")
                    with tc.tile_pool(name="p3", bufs=1) as p3, \
                         tc.tile_pool(name="p3s", bufs=2) as p3s, \
                         tc.tile_pool(name="psP", bufs=2, space="PSUM") as psP, \
                         tc.tile_pool(name="psT", bufs=1, space="PSUM") as psT:
                        y1_t = big.tile([128, DC, M], BF16, tag="slotL",
                                        name="y1")
                        psum_sum = psT.tile([128, M], F32, tag="s1",
                                            name="psum_sum")
                        psum_ssq = psT.tile([128, M], F32, tag="s2",
                                            name="psum_ssq")
                        for do in range(DC):
                            pp = psP.tile([128, M], F32, tag="pp", name="pp")
                            for di in range(DC):
                                nc.tensor.matmul(
                                    pp, wproj_t[:, di, do * 128:(do + 1) * 128],
                                    aT_t[:, di, :], start=(di == 0),
                                    stop=(di == DC - 1))
                            nc.vector.scalar_tensor_tensor(
                                y1_t[:, do, :], pp, consts["bprj"][:, do:do + 1],
                                xTq_t[:, do, :], op0=OP.add, op1=OP.add)
                            sq = p3s.tile([128, M], BF16, tag="sq", name="sq")
                            nc.vector.tensor_mul(sq, y1_t[:, do, :],
                                                 y1_t[:, do, :])
                            nc.tensor.matmul(psum_sum, consts["ones128"],
                                             y1_t[:, do, :],
                                             start=(do == 0), stop=(do == DC - 1))
                            nc.tensor.matmul(psum_ssq, consts["ones128"], sq,
                                             start=(do == 0), stop=(do == DC - 1))

                        # t1 = y1 - mu (bf16): the FFN contraction input.
                        # rstd is folded in per-f on the PSUM result, so the
                        # sqrt/reciprocal chain is off the FFN1 start path.
                        m_bc, rstd_bc = ln_stats_bc(p3, foldp,
                                                    psum_sum, psum_ssq)
                        with tc.tile_pool(name="lnx", bufs=2) as lnx:
                            for c2 in range(DC):
                                nc.vector.tensor_sub(t1T[:, c2, :],
                                                     y1_t[:, c2, :], m_bc)

                    # ------------ Phase 4: FFN + residual + LN2 ------------
                    with tc.tile_pool(name="p4a", bufs=2) as p4a, \
                         tc.tile_pool(name="p4h", bufs=2) as p4h, \
                         tc.tile_pool(name="psM", bufs=1, space="PSUM") as psM:
                        psm = [psM.tile([128, M], F32, tag=f"m{do}",
                                        name=f"psm{do}") for do in range(DC)]
                        with tc.tile_pool(name="lnx2", bufs=2) as lnx2, \
                             tc.tile_pool(name="psF", bufs=2,
                                          space="PSUM") as psF:
                            for f in range(FC):
                                wfcf = p4w.tile([128, DC, 128], BF16, tag="wfcf",
                                                name="wfcf")
                                nc.sync.dma_start(out=wfcf, in_=wfc[:, f])
                                woutf = p4w.tile([128, D], BF16, tag="woutf",
                                                 name="woutf")
                                nc.gpsimd.dma_start(
                                    out=woutf, in_=wout[f * 128:(f + 1) * 128, :])
                                pf = psF.tile([128, M], F32, tag="pf", name="pf")
                                for dc in range(DC):
                                    nc.tensor.matmul(pf, wfcf[:, dc, :],
                                                     t1T[:, dc, :],
                                                     start=(dc == 0),
                                                     stop=(dc == DC - 1))
                                nc.vector.tensor_mul(pf, pf, rstd_bc)
                                a1 = p4a.tile([128, M], BF16, tag="a1", name="a1")
                                nc.scalar.activation(a1, pf, AF.Gelu_apprx_tanh,
                                                     bias=consts["bfc"][:, f:f + 1])
                                for do in range(DC):
                                    nc.tensor.matmul(
                                        psm[do],
                                        woutf[:, do * 128:(do + 1) * 128],
                                        a1, start=(f == 0), stop=(f == FC - 1))
                                # nT = ((y1-mu)*rstd)*g + b from the fp32 y1
                                # (not the bf16 t1T), interleaved so DVE
                                # stays off the pf critical path
                                if f % 4 == 2 and f // 4 < DC:
                                    c2 = f // 4
                                    v1 = lnx2.tile([128, M], F32, tag="v1",
                                                   name="v1")
                                    nc.vector.tensor_sub(
                                        v1, y1_t[:, c2, :], m_bc)
                                    v2 = lnx2.tile([128, M], F32, tag="v2",
                                                   name="v2")
                                    nc.vector.tensor_mul(v2, v1, rstd_bc)
                                    nc.vector.tensor_scalar(
                                        nT_t[:, c2, :], v2,
                                        consts["l1g"][:, c2:c2 + 1],
                                        consts["l1b"][:, c2:c2 + 1],
                                        op0=OP.mult, op1=OP.add)

                        with tc.tile_pool(name="psT2", bufs=1,
                                          space="PSUM") as psT2:
                            y2_t = big.tile([128, DC, M], BF16,
                                            tag="slotM", name="y2")
                            psum_sum2 = psT2.tile([128, M], F32, tag="s1",
                                                  name="psum_sum2")
                            psum_ssq2 = psT2.tile([128, M], F32, tag="s2",
                                                  name="psum_ssq2")
                            for do in range(DC):
                                nc.vector.scalar_tensor_tensor(
                                    y2_t[:, do, :], psm[do],
                                    consts["bout"][:, do:do + 1],
                                    nT_t[:, do, :],
                                    op0=OP.add, op1=OP.add)
                                sq = p4a.tile([128, M], BF16, tag="sq2",
                                              name="sq2")
                                nc.vector.tensor_mul(
                                    sq, y2_t[:, do, :], y2_t[:, do, :])
                                nc.tensor.matmul(psum_sum2, consts["ones128"],
                                                 y2_t[:, do, :],
                                                 start=(do == 0),
                                                 stop=(do == DC - 1))
                                nc.tensor.matmul(psum_ssq2, consts["ones128"], sq,
                                                 start=(do == 0),
                                                 stop=(do == DC - 1))

                            m2_bc, rstd2_bc = ln_stats_bc(p4h, p4h,
                                                          psum_sum2, psum_ssq2,
                                                          sdt=BF16)
                            # padded-query rows are NOT zeroed here; the host
                            # zeroes them exactly during unsharding
                            with tc.tile_pool(name="lnz", bufs=2) as lnz:
                                for c2 in range(DC):
                                    u1 = lnz.tile([128, M], BF16, tag="u1",
                                                  name="u1")
                                    nc.vector.tensor_sub(
                                        u1, y2_t[:, c2, :], m2_bc)
                                    u2 = lnz.tile([128, M], BF16, tag="u2",
                                                  name="u2")
                                    nc.vector.tensor_mul(u2, u1, rstd2_bc)
                                    hc = lnz.tile([128, M], F32, tag="hc",
                                                  name="hc")
                                    nc.vector.tensor_scalar(
                                        hc, u2, consts["l2g"][:, c2:c2 + 1],
                                        consts["l2b"][:, c2:c2 + 1],
                                        op0=OP.mult, op1=OP.add)
                                    nc.scalar.dma_start(out=hT_r[c2], in_=hc)

    nc.compile()
    return nc


def _to_bf16(a):
    import ml_dtypes
    return np.asarray(a, dtype=np.float32).astype(ml_dtypes.bfloat16)


def _pack_w(w, n_out):
    # [D_in, N_out] -> [128, N_out/128, D_in/128, 128]: one contiguous line
    # per partition per out-chunk
    d_in = w.shape[0]
    return np.ascontiguousarray(
        w.reshape(d_in // 128, 128, n_out // 128, 128).transpose(1, 2, 0, 3))


def _shared_arrays(inputs):
    f32 = np.float32
    w_qkv = np.ascontiguousarray(inputs["w_qkv"], dtype=f32)
    b_qkv = np.ascontiguousarray(inputs["b_qkv"], dtype=f32)

    def pc(v):  # [C*128] -> [128, C] column-chunk layout
        v = np.ascontiguousarray(v, dtype=f32)
        return np.ascontiguousarray(v.reshape(-1, 128).T)

    w_fc_raw = np.ascontiguousarray(inputs["w_fc"], dtype=np.float64)
    ln1_g = np.asarray(inputs["ln1_g"], dtype=np.float64)
    ln1_b = np.asarray(inputs["ln1_b"], dtype=np.float64)
    wfcg = (w_fc_raw * ln1_g[:, None]).astype(f32)
    # gelu bias: b_fc + w_fc^T ln1_b (the +b part of LN1 folded out of the
    # FFN contraction)
    cb = (np.asarray(inputs["b_fc"], dtype=np.float64)
          + (w_fc_raw * ln1_b[:, None]).sum(axis=0)).astype(f32)
    wv_ext = np.zeros((D, VW), f32)
    bv_ext = np.zeros((VW,), f32)
    for h in range(H):
        wv_ext[:, h * (DH + 1):h * (DH + 1) + DH] = \
            w_qkv[:, 2 * D + h * DH:2 * D + (h + 1) * DH]
        bv_ext[h * (DH + 1):h * (DH + 1) + DH] = \
            b_qkv[2 * D + h * DH:2 * D + (h + 1) * DH]
        bv_ext[h * (DH + 1) + DH] = 1.0

    return dict(
        wq=_to_bf16(_pack_w(w_qkv[:, 0:D], D)),
        bq_pc=pc(b_qkv[0:D]),
        wk=_to_bf16(_pack_w(w_qkv[:, D:2 * D], D)),
        bk_pc=pc(b_qkv[D:2 * D]),
        wv=_to_bf16(wv_ext),
        bvb=_to_bf16(np.broadcast_to(bv_ext[None, :], (128, VW))),
        wproj=_to_bf16(np.asarray(inputs["w_proj"], dtype=f32)),
        wfc=_to_bf16(_pack_w(wfcg, FF)),
        wout=_to_bf16(np.asarray(inputs["w_out"], dtype=f32)),
        epsc=np.full((128, 1), EPS, f32),
        ones128=_to_bf16(np.ones((128, 128), f32)),
        bprj=pc(inputs["b_proj"]),
        bfc=pc(cb),
        bout=pc(inputs["b_out"]),
        l1g=pc(inputs["ln1_g"]),
        l1b=pc(inputs["ln1_b"]),
        l2g=pc(inputs["ln2_g"]),
        l2b=pc(inputs["ln2_b"]),
    )


def make_in_maps(inputs):
    inputs = {k: np.asarray(v) for k, v in inputs.items()}
    x = np.ascontiguousarray(inputs["x"], dtype=np.float32)
    lengths = np.asarray(inputs["lengths"]).astype(np.int64)
    shared = _shared_arrays(inputs)
    pos = np.arange(T)
    in_maps = []
    for c in range(8):
        b, r = divmod(c, 4)
        sl = slice(r * M, (r + 1) * M)
        xTb = _to_bf16(x[b].T)
        km = (pos < lengths[b]).astype(np.float32)
        m = dict(shared)
        m["xT"] = xTb
        m["xTq"] = np.ascontiguousarray(xTb[:, sl])
        m["vmask"] = np.ascontiguousarray(km.reshape(NKC, 128).T)
        in_maps.append(m)
    return in_maps


def get_program(reps=1):
    key = f"nc{reps}"
    if key not in _STATE:
        _STATE[key] = _build_program(reps)
    return _STATE[key]


def kernel(**inputs) -> np.ndarray:
    from concourse.bass_utils import run_bass_kernel_spmd

    nc = get_program()
    in_maps = make_in_maps(inputs)
    res = run_bass_kernel_spmd(nc, in_maps, list(range(8)), trace=False)
    out = np.zeros((B, T, D), np.float32)
    for c in range(8):
        b, r = divmod(c, 4)
        out[b, r * M:(r + 1) * M, :] = res.results[c]["hT"].T
    # zero padded-query rows exactly (the reference's final row mask)
    lengths = np.asarray(inputs["lengths"]).astype(np.int64)
    for b in range(B):
        out[b, lengths[b]:, :] = 0.0
    return out
